# revision 9
# baseline (speedup 1.0000x reference)
"""CFAR box-filter kernel (31x31 / 11x11 box sums + ratio) for Trainium2.

Data-parallel over batch: 32 images -> 8 NeuronCores, 4 images each.
Per 128-row chunk:
  - horizontal prefix scan (DVE) over zero-padded rows; window sums by
    shifted-column subtraction (h31 on GPSIMD, h11 on DVE),
  - vertical box sums as banded fp32r matmuls (weights carry the 1/121
    and +-1/840 output scales; halo rows gathered by SBUF-SBUF DMA),
  - front lands in PSUM (ACT copies it out), back lands in PSUM
    (DVE fast reciprocal, GPSIMD multiply -> ratio).
"""

import os
import sys

import numpy as np

for _p in ("/opt/trn_rl_repo", "/root/.axon_site/_ro/trn_rl_repo"):
    if os.path.isdir(_p) and _p not in sys.path:
        sys.path.insert(0, _p)
        break

import concourse.bass as bass
import concourse.tile as tile
from concourse import bacc
from concourse import mybir
from concourse._compat import with_exitstack
from concourse.bass_utils import run_bass_kernel_spmd

B, H, W = 32, 1024, 1024
NCORES = 8
BPC = B // NCORES            # images per core
CHUNKS = H // 128            # row chunks per image
PADL, PADR = 16, 15
SCANW = PADL + W + PADR      # 1055
XW = 2 * SCANW               # two chunks per x-load DMA
F32 = mybir.dt.float32
F32R = mybir.dt.float32r

GUARD, BG = 5, 10
R_IN = GUARD                 # 11x11 radius
R_OUT = GUARD + BG           # 31x31 radius
AREA_FRONT = float((2 * R_IN + 1) ** 2)                       # 121
AREA_BACK = float((2 * R_OUT + 1) ** 2 - (2 * R_IN + 1) ** 2)  # 840


def _weights() -> dict[str, np.ndarray]:
    k = np.arange(128)[:, None]
    m = np.arange(128)[None, :]
    g11 = np.arange(2 * R_IN)[:, None]
    g11 = np.where(g11 < R_IN, g11 - R_IN, 128 + (g11 - R_IN))
    g31 = np.arange(2 * R_OUT)[:, None]
    g31 = np.where(g31 < R_OUT, g31 - R_OUT, 128 + (g31 - R_OUT))

    def band(gg, radius, scale):
        w = ((np.abs(gg - m) <= radius) * scale).astype(np.float32)
        bits = w.view(np.uint32)
        bits = (bits + 0x1000) & np.uint32(0xFFFFE000)  # round to TF32
        return bits.view(np.float32).copy()

    w = {
        "wf_blk": band(k, R_IN, 1.0 / AREA_FRONT),
        "wf_halo": band(g11, R_IN, 1.0 / AREA_FRONT),
        "wb31_blk": band(k, R_OUT, 1.0 / AREA_BACK),
        "wb31_halo": band(g31, R_OUT, 1.0 / AREA_BACK),
        "wn11_blk": band(k, R_IN, -1.0 / AREA_BACK),
        "wn11_halo": band(g11, R_IN, -1.0 / AREA_BACK),
    }
    # prev/next-only halo slices for the image's edge chunks (no zero rows)
    for nm, r in (("wf_halo", R_IN), ("wb31_halo", R_OUT), ("wn11_halo", R_IN)):
        w[nm + "P"] = w[nm][:r].copy()
        w[nm + "N"] = w[nm][r:].copy()
    return w


@with_exitstack
def _cfar_tile_kernel(ctx, tc, x_d, o_d, w_d, n_img):
    nc = tc.nc
    ADD = mybir.AluOpType.add
    BYP = mybir.AluOpType.bypass

    const = ctx.enter_context(tc.tile_pool(name="const", bufs=1))
    wt = {}
    for name, dram_ap in w_d.items():
        t = const.tile(list(dram_ap.shape), F32R, tag=name)
        nc.sync.dma_start(t[:], dram_ap)
        wt[name] = t

    xp = ctx.enter_context(tc.tile_pool(name="xp", bufs=2))
    cp = ctx.enter_context(tc.tile_pool(name="cp", bufs=3))
    h31p = ctx.enter_context(tc.tile_pool(name="h31p", bufs=5))
    h11p = ctx.enter_context(tc.tile_pool(name="h11p", bufs=5))
    gp = ctx.enter_context(tc.tile_pool(name="gp", bufs=3))
    pp = ctx.enter_context(tc.tile_pool(name="pp", bufs=2, space="PSUM"))
    rp = ctx.enter_context(tc.tile_pool(name="rp", bufs=3))
    obp = ctx.enter_context(tc.tile_pool(name="obp", bufs=2))

    for img in range(n_img):
        h31s: dict[int, object] = {}
        h11s: dict[int, object] = {}
        ob0: dict[int, object] = {}
        ob1: dict[int, object] = {}

        def produce(tb):
            xt = xp.tile([128, XW], F32, tag="xt")
            src = x_d[img, 256 * tb : 256 * (tb + 1), :].rearrange(
                "(c p) w -> p c w", c=2
            )
            dst = xt[:].rearrange("p (c s) -> p c s", c=2)
            nc.sync.dma_start(dst, src)
            for c in (0, 1):
                t = 2 * tb + c
                C = cp.tile([128, SCANW], F32, tag="C")
                xs = xt[:, c * SCANW : (c + 1) * SCANW]
                nc.vector.tensor_tensor_scan(C[:], xs, xs, 0.0, ADD, BYP)
                h31 = h31p.tile([128, W], F32R, tag="h31")
                nc.gpsimd.tensor_sub(
                    h31[:], C[:, 2 * R_OUT + 1 : 2 * R_OUT + 1 + W], C[:, 0:W]
                )
                h11 = h11p.tile([128, W], F32R, tag="h11")
                nc.vector.tensor_sub(
                    h11[:],
                    C[:, PADL + R_IN : PADL + R_IN + W],
                    C[:, PADL - R_IN - 1 : PADL - R_IN - 1 + W],
                )
                h31s[t] = h31
                h11s[t] = h11

        def consume(t):
            p, c = divmod(t, 2)
            if t == 0:
                g31 = gp.tile([R_OUT, W], F32R, tag="g31")
                g11 = gp.tile([R_IN, W], F32R, tag="g11")
                nc.sync.dma_start(g31[:], h31s[1][0:R_OUT, :])
                nc.sync.dma_start(g11[:], h11s[1][0:R_IN, :])
                sfx = "N"
            elif t == CHUNKS - 1:
                g31 = gp.tile([R_OUT, W], F32R, tag="g31")
                g11 = gp.tile([R_IN, W], F32R, tag="g11")
                nc.sync.dma_start(g31[:], h31s[t - 1][128 - R_OUT : 128, :])
                nc.sync.dma_start(g11[:], h11s[t - 1][128 - R_IN : 128, :])
                sfx = "P"
            else:
                g31 = gp.tile([2 * R_OUT, W], F32R, tag="g31")
                g11 = gp.tile([2 * R_IN, W], F32R, tag="g11")
                nc.sync.dma_start(g31[0:R_OUT, :], h31s[t - 1][128 - R_OUT : 128, :])
                nc.sync.dma_start(g11[0:R_IN, :], h11s[t - 1][128 - R_IN : 128, :])
                nc.sync.dma_start(g31[R_OUT : 2 * R_OUT, :], h31s[t + 1][0:R_OUT, :])
                nc.sync.dma_start(g11[R_IN : 2 * R_IN, :], h11s[t + 1][0:R_IN, :])
                sfx = ""

            psf = pp.tile([128, W], F32, tag="front")
            psb = pp.tile([128, W], F32, tag="back")
            MM = nc.tensor.matmul
            for h0 in (0, 512):
                s = slice(h0, h0 + 512)
                MM(psf[:, s], wt["wf_blk"][:],
                   h11s[t][:, s], start=True, stop=False)
                MM(psf[:, s], wt["wf_halo" + sfx][:],
                   g11[:, s], start=False, stop=True)
                MM(psb[:, s], wt["wb31_blk"][:],
                   h31s[t][:, s], start=True, stop=False)
                MM(psb[:, s], wt["wb31_halo" + sfx][:],
                   g31[:, s], start=False, stop=False)
                MM(psb[:, s], wt["wn11_blk"][:],
                   h11s[t][:, s], start=False, stop=False)
                MM(psb[:, s], wt["wn11_halo" + sfx][:],
                   g11[:, s], start=False, stop=True)

            if c == 0:
                ob0[p] = obp.tile([128, 2 * W], F32, tag="ob0", name=f"ob0_{img}_{p}")
                ob1[p] = obp.tile([128, 2 * W], F32, tag="ob1", name=f"ob1_{img}_{p}")
            o0 = ob0[p][:, c * W : (c + 1) * W]
            o1 = ob1[p][:, c * W : (c + 1) * W]
            r = rp.tile([128, W], F32, tag="r")
            nc.vector.reciprocal_approx_fast(out=r[:], in_=psb[:])
            nc.scalar.copy(o1, psf[:])
            nc.gpsimd.tensor_mul(o0, o1, r[:])
            if c == 1:
                d0 = o_d[img, 256 * p : 256 * (p + 1), :].rearrange(
                    "(c q) w -> q c w", c=2
                )
                d1 = o_d[n_img + img, 256 * p : 256 * (p + 1), :].rearrange(
                    "(c q) w -> q c w", c=2
                )
                nc.scalar.dma_start(d0, ob0[p][:].rearrange("q (c w) -> q c w", c=2))
                nc.scalar.dma_start(d1, ob1[p][:].rearrange("q (c w) -> q c w", c=2))

        produce(0)
        consume(0)
        for tb in range(1, CHUNKS // 2):
            produce(tb)
            consume(2 * tb - 1)
            consume(2 * tb)
        consume(CHUNKS - 1)


def build(n_img: int = BPC):
    nc = bacc.Bacc("TRN2", target_bir_lowering=False, debug=False)
    x_d = nc.dram_tensor("x", [n_img, H, SCANW], F32, kind="ExternalInput").ap()
    o_d = nc.dram_tensor("out", [2 * n_img, H, W], F32, kind="ExternalOutput").ap()
    wts = _weights()
    w_d = {
        k: nc.dram_tensor(k, list(v.shape), F32R, kind="ExternalInput").ap()
        for k, v in wts.items()
    }
    with tile.TileContext(nc) as tc:
        _cfar_tile_kernel(tc, x_d, o_d, w_d, n_img)
    nc.compile()
    return nc, wts


_CACHE: dict = {}


def kernel(x: np.ndarray) -> np.ndarray:
    x = np.ascontiguousarray(np.asarray(x, dtype=np.float32))
    assert x.shape == (B, 1, H, W), x.shape
    if "nc" not in _CACHE:
        _CACHE["nc"], _CACHE["wts"] = build(BPC)
    nc, wts = _CACHE["nc"], _CACHE["wts"]
    xs = np.zeros((B, H, SCANW), dtype=np.float32)
    xs[:, :, PADL : PADL + W] = x[:, 0]
    in_maps = []
    for i in range(NCORES):
        m = {"x": np.ascontiguousarray(xs[BPC * i : BPC * (i + 1)])}
        m.update(wts)
        in_maps.append(m)
    res = run_bass_kernel_spmd(nc, in_maps, list(range(NCORES))).results
    out = np.empty((2 * B, 1, H, W), dtype=np.float32)
    for i in range(NCORES):
        o = res[i]["out"]
        out[BPC * i : BPC * (i + 1), 0] = o[:BPC]
        out[B + BPC * i : B + BPC * (i + 1), 0] = o[BPC:]
    return out


# revision 11
# speedup vs baseline: 6061.9028x; 6061.9028x over previous
"""CFAR box-filter kernel (31x31 / 11x11 box sums + ratio) for Trainium2.

Data-parallel over batch: 32 images -> 8 NeuronCores, 4 images each.
Per 128-row chunk:
  - horizontal prefix scan (DVE) over zero-padded rows; window sums by
    shifted-column subtraction (h31 on GPSIMD, h11 on DVE),
  - vertical box sums as banded fp32r matmuls (weights carry the 1/121
    and +-1/840 output scales; halo rows gathered by SBUF-SBUF DMA),
  - front lands in PSUM (ACT copies it out), back lands in PSUM
    (DVE fast reciprocal, GPSIMD multiply -> ratio).
"""

import os
import sys

import numpy as np

for _p in ("/opt/trn_rl_repo", "/root/.axon_site/_ro/trn_rl_repo"):
    if os.path.isdir(_p) and _p not in sys.path:
        sys.path.insert(0, _p)
        break

import concourse.bass as bass
import concourse.tile as tile
from concourse import bacc
from concourse import mybir
from concourse._compat import with_exitstack
from concourse.bass_utils import run_bass_kernel_spmd

B, H, W = 32, 1024, 1024
NCORES = 8
BPC = B // NCORES            # images per core
CHUNKS = H // 128            # row chunks per image
PADL, PADR = 16, 15
SCANW = PADL + W + PADR      # 1055
XW = 2 * SCANW               # two chunks per x-load DMA
F32 = mybir.dt.float32
F32R = mybir.dt.float32r

GUARD, BG = 5, 10
R_IN = GUARD                 # 11x11 radius
R_OUT = GUARD + BG           # 31x31 radius
AREA_FRONT = float((2 * R_IN + 1) ** 2)                       # 121
AREA_BACK = float((2 * R_OUT + 1) ** 2 - (2 * R_IN + 1) ** 2)  # 840


def _weights() -> dict[str, np.ndarray]:
    k = np.arange(128)[:, None]
    m = np.arange(128)[None, :]
    g11 = np.arange(2 * R_IN)[:, None]
    g11 = np.where(g11 < R_IN, g11 - R_IN, 128 + (g11 - R_IN))
    g31 = np.arange(2 * R_OUT)[:, None]
    g31 = np.where(g31 < R_OUT, g31 - R_OUT, 128 + (g31 - R_OUT))

    def band(gg, radius, scale):
        w = ((np.abs(gg - m) <= radius) * scale).astype(np.float32)
        bits = w.view(np.uint32)
        bits = (bits + 0x1000) & np.uint32(0xFFFFE000)  # round to TF32
        return bits.view(np.float32).copy()

    w = {
        "wf_blk": band(k, R_IN, 1.0 / AREA_FRONT),
        "wf_halo": band(g11, R_IN, 1.0 / AREA_FRONT),
        "wb31_blk": band(k, R_OUT, 1.0 / AREA_BACK),
        "wb31_halo": band(g31, R_OUT, 1.0 / AREA_BACK),
        "wn11_blk": band(k, R_IN, -1.0 / AREA_BACK),
        "wn11_halo": band(g11, R_IN, -1.0 / AREA_BACK),
    }
    # prev/next-only halo slices for the image's edge chunks (no zero rows)
    for nm, r in (("wf_halo", R_IN), ("wb31_halo", R_OUT), ("wn11_halo", R_IN)):
        w[nm + "P"] = w[nm][:r].copy()
        w[nm + "N"] = w[nm][r:].copy()
    return w


@with_exitstack
def _cfar_tile_kernel(ctx, tc, x_d, o_d, w_d, n_img, reps=1):
    nc = tc.nc
    ADD = mybir.AluOpType.add
    BYP = mybir.AluOpType.bypass

    const = ctx.enter_context(tc.tile_pool(name="const", bufs=1))
    wt = {}
    for name, dram_ap in w_d.items():
        t = const.tile(list(dram_ap.shape), F32R, tag=name)
        nc.sync.dma_start(t[:], dram_ap)
        wt[name] = t

    xp = ctx.enter_context(tc.tile_pool(name="xp", bufs=2))
    cp = ctx.enter_context(tc.tile_pool(name="cp", bufs=3))
    h31p = ctx.enter_context(tc.tile_pool(name="h31p", bufs=5))
    h11p = ctx.enter_context(tc.tile_pool(name="h11p", bufs=5))
    gp = ctx.enter_context(tc.tile_pool(name="gp", bufs=3))
    pp = ctx.enter_context(tc.tile_pool(name="pp", bufs=2, space="PSUM"))
    rp = ctx.enter_context(tc.tile_pool(name="rp", bufs=3))
    obp = ctx.enter_context(tc.tile_pool(name="obp", bufs=2))

    def one_pass():
      for img in range(n_img):
        h31s: dict[int, object] = {}
        h11s: dict[int, object] = {}
        ob0: dict[int, object] = {}
        ob1: dict[int, object] = {}

        def produce(tb):
            xt = xp.tile([128, XW], F32, tag="xt")
            src = x_d[img, 256 * tb : 256 * (tb + 1), :].rearrange(
                "(c p) w -> p c w", c=2
            )
            dst = xt[:].rearrange("p (c s) -> p c s", c=2)
            nc.sync.dma_start(dst, src)
            for c in (0, 1):
                t = 2 * tb + c
                C = cp.tile([128, SCANW], F32, tag="C")
                xs = xt[:, c * SCANW : (c + 1) * SCANW]
                nc.vector.tensor_tensor_scan(C[:], xs, xs, 0.0, ADD, BYP)
                h31 = h31p.tile([128, W], F32R, tag="h31")
                nc.gpsimd.tensor_sub(
                    h31[:], C[:, 2 * R_OUT + 1 : 2 * R_OUT + 1 + W], C[:, 0:W]
                )
                h11 = h11p.tile([128, W], F32R, tag="h11")
                nc.vector.tensor_sub(
                    h11[:],
                    C[:, PADL + R_IN : PADL + R_IN + W],
                    C[:, PADL - R_IN - 1 : PADL - R_IN - 1 + W],
                )
                h31s[t] = h31
                h11s[t] = h11

        def consume(t):
            p, c = divmod(t, 2)
            if t == 0:
                g31 = gp.tile([R_OUT, W], F32R, tag="g31")
                g11 = gp.tile([R_IN, W], F32R, tag="g11")
                nc.sync.dma_start(g31[:], h31s[1][0:R_OUT, :])
                nc.sync.dma_start(g11[:], h11s[1][0:R_IN, :])
                sfx = "N"
            elif t == CHUNKS - 1:
                g31 = gp.tile([R_OUT, W], F32R, tag="g31")
                g11 = gp.tile([R_IN, W], F32R, tag="g11")
                nc.sync.dma_start(g31[:], h31s[t - 1][128 - R_OUT : 128, :])
                nc.sync.dma_start(g11[:], h11s[t - 1][128 - R_IN : 128, :])
                sfx = "P"
            else:
                g31 = gp.tile([2 * R_OUT, W], F32R, tag="g31")
                g11 = gp.tile([2 * R_IN, W], F32R, tag="g11")
                nc.sync.dma_start(g31[0:R_OUT, :], h31s[t - 1][128 - R_OUT : 128, :])
                nc.sync.dma_start(g11[0:R_IN, :], h11s[t - 1][128 - R_IN : 128, :])
                nc.sync.dma_start(g31[R_OUT : 2 * R_OUT, :], h31s[t + 1][0:R_OUT, :])
                nc.sync.dma_start(g11[R_IN : 2 * R_IN, :], h11s[t + 1][0:R_IN, :])
                sfx = ""

            psf = pp.tile([128, W], F32, tag="front")
            psb = pp.tile([128, W], F32, tag="back")
            MM = nc.tensor.matmul
            for h0 in (0, 512):
                s = slice(h0, h0 + 512)
                MM(psf[:, s], wt["wf_blk"][:],
                   h11s[t][:, s], start=True, stop=False)
                MM(psf[:, s], wt["wf_halo" + sfx][:],
                   g11[:, s], start=False, stop=True)
                MM(psb[:, s], wt["wb31_blk"][:],
                   h31s[t][:, s], start=True, stop=False)
                MM(psb[:, s], wt["wb31_halo" + sfx][:],
                   g31[:, s], start=False, stop=False)
                MM(psb[:, s], wt["wn11_blk"][:],
                   h11s[t][:, s], start=False, stop=False)
                MM(psb[:, s], wt["wn11_halo" + sfx][:],
                   g11[:, s], start=False, stop=True)

            if c == 0:
                ob0[p] = obp.tile([128, 2 * W], F32, tag="ob0", name=f"ob0_{img}_{p}")
                ob1[p] = obp.tile([128, 2 * W], F32, tag="ob1", name=f"ob1_{img}_{p}")
            o0 = ob0[p][:, c * W : (c + 1) * W]
            o1 = ob1[p][:, c * W : (c + 1) * W]
            r = rp.tile([128, W], F32, tag="r")
            nc.vector.reciprocal_approx_fast(out=r[:], in_=psb[:])
            nc.scalar.copy(o1, psf[:])
            nc.gpsimd.tensor_mul(o0, o1, r[:])
            if c == 1:
                d0 = o_d[img, 256 * p : 256 * (p + 1), :].rearrange(
                    "(c q) w -> q c w", c=2
                )
                d1 = o_d[n_img + img, 256 * p : 256 * (p + 1), :].rearrange(
                    "(c q) w -> q c w", c=2
                )
                nc.scalar.dma_start(d0, ob0[p][:].rearrange("q (c w) -> q c w", c=2))
                nc.scalar.dma_start(d1, ob1[p][:].rearrange("q (c w) -> q c w", c=2))

        produce(0)
        consume(0)
        for tb in range(1, CHUNKS // 2):
            produce(tb)
            consume(2 * tb - 1)
            consume(2 * tb)
        consume(CHUNKS - 1)

    if reps == 1:
        one_pass()
    else:
        with tc.For_i(0, reps, 1):
            one_pass()


def build(n_img: int = BPC, reps: int = 1):
    nc = bacc.Bacc("TRN2", target_bir_lowering=False, debug=False)
    x_d = nc.dram_tensor("x", [n_img, H, SCANW], F32, kind="ExternalInput").ap()
    o_d = nc.dram_tensor("out", [2 * n_img, H, W], F32, kind="ExternalOutput").ap()
    wts = _weights()
    w_d = {
        k: nc.dram_tensor(k, list(v.shape), F32R, kind="ExternalInput").ap()
        for k, v in wts.items()
    }
    with tile.TileContext(nc) as tc:
        _cfar_tile_kernel(tc, x_d, o_d, w_d, n_img, reps)
    nc.compile()
    return nc, wts


_CACHE: dict = {}


def kernel(x: np.ndarray) -> np.ndarray:
    x = np.ascontiguousarray(np.asarray(x, dtype=np.float32))
    assert x.shape == (B, 1, H, W), x.shape
    if "nc" not in _CACHE:
        _CACHE["nc"], _CACHE["wts"] = build(BPC)
    nc, wts = _CACHE["nc"], _CACHE["wts"]
    xs = np.zeros((B, H, SCANW), dtype=np.float32)
    xs[:, :, PADL : PADL + W] = x[:, 0]
    in_maps = []
    for i in range(NCORES):
        m = {"x": np.ascontiguousarray(xs[BPC * i : BPC * (i + 1)])}
        m.update(wts)
        in_maps.append(m)
    res = run_bass_kernel_spmd(nc, in_maps, list(range(NCORES))).results
    out = np.empty((2 * B, 1, H, W), dtype=np.float32)
    for i in range(NCORES):
        o = res[i]["out"]
        out[BPC * i : BPC * (i + 1), 0] = o[:BPC]
        out[B + BPC * i : B + BPC * (i + 1), 0] = o[BPC:]
    return out
